# revision 20
# baseline (speedup 1.0000x reference)
"""Trainium2 Bass kernel for top-1 MoE expert MLP (nn_Experts problem).

Strategy (expert-parallel, one expert per NeuronCore):
  - Routing is one-hot top-1: each token is processed by exactly one expert,
    so each core computes the MLP only for the tokens routed to its expert.
  - Capacity CAP=512 = T/E: every core does identical, perfectly balanced
    work (16+8 psum tiles, moving dim 512 = one full PSUM bank).  The few
    tokens beyond an expert's capacity (92 of 4096 for the reference
    routing) take the exact-fp32 host fallback.
  - All matmul operands are bf16 (PSUM accumulates fp32): halves HBM/DMA
    traffic vs fp32/fp32r.  End-to-end rel err ~4e-3 vs the 2e-2 gate.
  - Phase A: h^T[F, CAP] = gelu(w1^T @ x^T + b1), h stored bf16.
  - Phase B computes y^T[D, CAP] = w2^T @ h^T (w2 in natural [F, D] layout
    is the stationary operand) so the moving dim is CAP=512, not D=1024:
    total PE rows = 16*8*512 + 8*16*512 = 131072 ~= 54.6us at 2.4 GHz.
  - The combine gate and b2 are applied on the host (pure elementwise on
    the gathered output), so the device does matmuls + gelu only.
  - All weight/x blocks are packed PARTITION-MAJOR on the host so every
    DMA descriptor is one contiguous multi-KB run per partition (2KB
    descriptors only reach ~205 GB/s effective; 6-8KB reach ~320 GB/s).
  - DMA issue is spread across queues (sync: head/x + y-out, scalar: w1
    then w2 -- per-ring FIFO keeps w2's 4MB from delaying w1, gpsimd: gb).
  - The PE p-state ramps 0.65->1.2->2.4 GHz over ~3us of continuous busy;
    a memset + 8 dummy matmuls at kernel start burn the ramp while the
    head DMA is still in flight, so real matmuls run at full clock.
"""

import numpy as np

B, N, D, E, F = 8, 512, 1024, 8, 2048
T = B * N
P = 128
CAP = 512            # per-expert token capacity = T/E (exact balance)
KT1 = D // P         # 8  k-tiles for matmul1 (contract over D)
MT1 = F // P         # 16 m-tiles for matmul1 / k-tiles for matmul2
MT2 = D // P         # 8  m-tiles for matmul2 (y^T rows)

W1_HEAD = 2                       # w1 m1-tiles packed into the head DMA
W1_BLOCKS = (2, 3, 4, 5)          # m1 = 2..15, all >=4KB descriptors
W2_BLK = 4                        # k2-tiles per w2 DMA
X_BLOCKS = (3, 4)                 # k = 1..7
N_DUMMY = 8                       # pre-ramp matmuls

_NC_CACHE = {}


def _build_bass():
    import concourse.bacc as bacc
    import concourse.tile as tile
    from concourse import mybir

    f32 = mybir.dt.float32
    bf16 = mybir.dt.bfloat16

    nc = bacc.Bacc(None, target_bir_lowering=False)
    # head: x k-tile 0 + w1 m1=0..W1_HEAD-1 column blocks in one transfer
    head = nc.declare_dram_parameter("head", [P, CAP + W1_HEAD * D], bf16,
                                     isOutput=False)
    gb = nc.declare_dram_parameter("gb", [P, MT1], f32, isOutput=False)
    # partition-major packs: row p holds that partition's full payload
    xb = nc.declare_dram_parameter("xb", [P, (KT1 - 1) * CAP], bf16,
                                   isOutput=False)
    w1b = nc.declare_dram_parameter("w1b", [P, (MT1 - W1_HEAD) * D], bf16,
                                    isOutput=False)
    w2b = nc.declare_dram_parameter("w2b", [P, MT1 * D], bf16,
                                    isOutput=False)
    yT = nc.declare_dram_parameter("yT", [D, CAP], bf16, isOutput=True)

    with tile.TileContext(nc) as tc:
        with (
            tc.tile_pool(name="gbp", bufs=1) as gbp,
            tc.tile_pool(name="hdp", bufs=1) as hdp,
            tc.tile_pool(name="dmp", bufs=1) as dmp,
            tc.tile_pool(name="xp", bufs=len(X_BLOCKS)) as xp,
            tc.tile_pool(name="w1p", bufs=len(W1_BLOCKS)) as w1p,
            tc.tile_pool(name="w2p", bufs=MT1 // W2_BLK) as w2p,
            tc.tile_pool(name="hp", bufs=MT1) as hp,
            tc.tile_pool(name="stp", bufs=4) as stp,
            tc.tile_pool(name="psA", bufs=3, space="PSUM") as psA,
            tc.tile_pool(name="psB", bufs=3, space="PSUM") as psB,
            tc.tile_pool(name="psD", bufs=1, space="PSUM") as psD,
        ):
            # ---- DMA issue -------------------------------------------
            # sync queue: head then x blocks (ring FIFO == arrival order)
            head_t = hdp.tile([P, CAP + W1_HEAD * D], bf16, tag="hd")
            nc.sync.dma_start(out=head_t[:], in_=head[:, :])
            x_blk, x_off = [], []
            off = 1
            for nk in X_BLOCKS:
                x_off.append(off)
                t = xp.tile([P, nk * CAP], bf16, tag="x",
                            name=f"x_{off}", padded_shape=[P, 4 * CAP])
                nc.sync.dma_start(
                    out=t[:], in_=xb[:, (off - 1) * CAP:(off - 1 + nk) * CAP])
                x_blk.append(t)
                off += nk

            def x_rhs(k):
                if k == 0:
                    return head_t[:, 0:CAP]
                j = next(i for i in range(len(X_BLOCKS))
                         if x_off[i] <= k < x_off[i] + X_BLOCKS[i])
                return x_blk[j][:, (k - x_off[j]) * CAP:(k - x_off[j] + 1) * CAP]

            # scalar queue: w1 m1=W1_HEAD..15 in growing blocks, THEN w2.
            w1_blk, w1_off = [], []
            off = W1_HEAD
            for nm in W1_BLOCKS:
                w1_off.append(off)
                t = w1p.tile([P, nm * D], bf16, tag="w1", name=f"w1_{off}",
                             padded_shape=[P, max(W1_BLOCKS) * D])
                nc.scalar.dma_start(
                    out=t[:], in_=w1b[:, (off - W1_HEAD) * D:
                                      (off - W1_HEAD + nm) * D])
                w1_blk.append(t)
                off += nm
            w2_sb = []
            for j in range(MT1 // W2_BLK):
                t = w2p.tile([P, W2_BLK * D], bf16, tag="w2", name=f"w2_{j}")
                nc.scalar.dma_start(
                    out=t[:], in_=w2b[:, j * W2_BLK * D:(j + 1) * W2_BLK * D])
                w2_sb.append(t)

            # gpsimd queue: bias columns
            gb_sb = gbp.tile([P, MT1], f32)
            nc.gpsimd.dma_start(out=gb_sb[:], in_=gb[:, :])

            def w1_lhs(m1, k):
                if m1 < W1_HEAD:
                    c0 = CAP + m1 * D + k * P
                    return head_t[:, c0:c0 + P]
                j = next(i for i in range(len(W1_BLOCKS))
                         if w1_off[i] <= m1 < w1_off[i] + W1_BLOCKS[i])
                c0 = (m1 - w1_off[j]) * D + k * P
                return w1_blk[j][:, c0:c0 + P]

            # ---- PE pre-ramp: burn the p-state ladder on junk ---------
            dum = dmp.tile([P, CAP], bf16, tag="dum")
            nc.vector.memset(dum[:], 0)
            psd = psD.tile([P, CAP], f32, tag="psD")
            for i in range(N_DUMMY):
                nc.tensor.matmul(psd[:], dum[:, 0:P], dum[:],
                                 start=True, stop=True, skip_group_check=True)

            # ---- Phase A: h^T[F, CAP] = gelu(w1^T @ x^T + b1) ----------
            gelu = mybir.ActivationFunctionType.Gelu
            h_sb = []
            for m1 in range(MT1):
                ps = psA.tile([P, CAP], f32, tag="psA", name=f"psA_{m1}")
                for k in range(KT1):
                    nc.tensor.matmul(ps[:], w1_lhs(m1, k), x_rhs(k),
                                     start=(k == 0), stop=(k == KT1 - 1))
                h = hp.tile([P, CAP], bf16, tag="h", name=f"h_{m1}")
                nc.scalar.activation(h[:], ps[:], gelu,
                                     bias=gb_sb[:, m1:m1 + 1])
                h_sb.append(h)

            # ---- Phase B: y^T[D, CAP] = w2^T @ h^T ---------------------
            # Last m-tile runs as two half-CAP groups so the final
            # copy+DMA chain after the very last matmul is half as long.
            def w2_lhs(k2, m):
                c0 = (k2 % W2_BLK) * D + m * P
                return w2_sb[k2 // W2_BLK][:, c0:c0 + P]

            for m in range(MT2 - 1):
                ps = psB.tile([P, CAP], f32, tag="psB", name=f"psB_{m}")
                for k2 in range(MT1):
                    nc.tensor.matmul(ps[:], w2_lhs(k2, m), h_sb[k2][:],
                                     start=(k2 == 0), stop=(k2 == MT1 - 1))
                stage = stp.tile([P, CAP], bf16, tag="st", name=f"st_{m}")
                nc.vector.tensor_scalar_mul(stage[:], ps[:], 1.0)
                nc.sync.dma_start(out=yT[m * P:(m + 1) * P, :],
                                  in_=stage[:])
            m = MT2 - 1
            for half, (a, b) in enumerate(((0, CAP // 2), (CAP // 2, CAP))):
                ps = psB.tile([P, b - a], f32, tag="psB", name=f"psB_{m}{half}")
                for k2 in range(MT1):
                    nc.tensor.matmul(ps[:], w2_lhs(k2, m), h_sb[k2][:, a:b],
                                     start=(k2 == 0), stop=(k2 == MT1 - 1))
                stage = stp.tile([P, b - a], bf16, tag="st",
                                 name=f"st_{m}{half}")
                nc.vector.tensor_scalar_mul(stage[:], ps[:], 1.0)
                nc.sync.dma_start(out=yT[m * P:(m + 1) * P, a:b],
                                  in_=stage[:])
    if not nc.is_finalized():
        nc.finalize()
    return nc


def _get_nc():
    if "nc" not in _NC_CACHE:
        _NC_CACHE["nc"] = _build_bass()
    return _NC_CACHE["nc"]


def kernel(x, dispatch_tensor, combine_tensor, w1, b1, w2, b2, **_):
    import ml_dtypes
    from concourse.bass_utils import run_bass_kernel_spmd

    bf = ml_dtypes.bfloat16
    x2d = np.ascontiguousarray(np.asarray(x, dtype=np.float32)).reshape(T, D)
    dispatch = np.asarray(dispatch_tensor, dtype=np.float32).reshape(T, E)
    combine = np.asarray(combine_tensor, dtype=np.float32).reshape(T, E)
    w1 = np.asarray(w1, dtype=np.float32)
    b1 = np.asarray(b1, dtype=np.float32)
    w2 = np.asarray(w2, dtype=np.float32)
    b2 = np.asarray(b2, dtype=np.float32)

    top = dispatch.argmax(-1)
    gate = combine.sum(-1)
    full = [np.nonzero(top == e)[0] for e in range(E)]
    idxs = [idx[:CAP] for idx in full]
    spill = [idx[CAP:] for idx in full]

    in_maps = []
    for e in range(E):
        idx = idxs[e]
        c = len(idx)
        xT = np.zeros((D, CAP), bf)
        xT[:, :c] = x2d[idx].T.astype(bf)
        # w1s[m1*P+p, k*P+m] = w1[k*P+p, m1*P+m]: per-m1 [P, D] blocks whose
        # [:, k*P:(k+1)*P] slice is the lhsT k-tile for output tile m1.
        w1s = np.ascontiguousarray(
            w1[e].reshape(KT1, P, MT1, P).transpose(2, 1, 0, 3)
        ).reshape(F, D).astype(bf)
        gbm = np.ascontiguousarray(b1[e].reshape(MT1, P).T)
        # partition-major packs: [n, P, w] -> [P, n*w]
        pm = lambda a: np.ascontiguousarray(
            a.transpose(1, 0, 2).reshape(P, -1))
        in_maps.append({
            "head": np.ascontiguousarray(np.concatenate(
                [xT[:P]] + [w1s[j * P:(j + 1) * P] for j in range(W1_HEAD)],
                axis=1)),
            "gb": gbm,
            "xb": pm(xT[P:].reshape(KT1 - 1, P, CAP)),
            "w1b": pm(w1s[W1_HEAD * P:].reshape(MT1 - W1_HEAD, P, D)),
            "w2b": pm(np.ascontiguousarray(w2[e]).astype(bf)
                      .reshape(MT1, P, D)),
        })

    global _LAST_IN_MAPS
    _LAST_IN_MAPS = in_maps
    nc = _get_nc()
    res = run_bass_kernel_spmd(nc, in_maps, list(range(E)))

    y_flat = np.empty((T, D), np.float32)
    for e in range(E):
        c = len(idxs[e])
        yTr = np.asarray(res.results[e]["yT"], dtype=np.float32)
        y_flat[idxs[e]] = yTr[:, :c].T * gate[idxs[e]][:, None]
        if len(spill[e]):
            # capacity-overflow fallback (exact fp32 math on host)
            import math

            erf = np.frompyfunc(math.erf, 1, 1)
            hs = x2d[spill[e]] @ w1[e] + b1[e]
            hs = hs * 0.5 * (1.0 + erf(hs / np.sqrt(2.0)).astype(np.float64))
            y_flat[spill[e]] = ((hs @ w2[e]) *
                                gate[spill[e]][:, None]).astype(np.float32)
    return (y_flat + b2[None, :]).reshape(B, N, D)


# revision 21
# speedup vs baseline: 1.0619x; 1.0619x over previous
"""Trainium2 Bass kernel for top-1 MoE expert MLP (nn_Experts problem).

Strategy (expert-parallel, one expert per NeuronCore):
  - Routing is one-hot top-1: each token is processed by exactly one expert,
    so each core computes the MLP only for the tokens routed to its expert.
  - Capacity CAP=512 = T/E: every core does identical, perfectly balanced
    work (16+8 psum tiles, moving dim 512 = one full PSUM bank).  The few
    tokens beyond an expert's capacity (92 of 4096 for the reference
    routing) take the exact-fp32 host fallback.
  - All matmul operands are bf16 (PSUM accumulates fp32): halves HBM/DMA
    traffic vs fp32/fp32r.  End-to-end rel err ~4e-3 vs the 2e-2 gate.
  - Phase A: h^T[F, CAP] = gelu(w1^T @ x^T + b1), h stored bf16.
  - Phase B computes y^T[D, CAP] = w2^T @ h^T (w2 in natural [F, D] layout
    is the stationary operand) so the moving dim is CAP=512, not D=1024:
    total PE rows = 16*8*512 + 8*16*512 = 131072 ~= 54.6us at 2.4 GHz.
  - The combine gate and b2 are applied on the host (pure elementwise on
    the gathered output), so the device does matmuls + gelu only.
  - All weight/x blocks are packed PARTITION-MAJOR on the host so every
    DMA descriptor is one contiguous multi-KB run per partition (2KB
    descriptors only reach ~205 GB/s effective; 6-8KB reach ~320 GB/s).
  - DMA issue is spread across queues (sync: head/x + y-out, scalar: w1
    then w2 -- per-ring FIFO keeps w2's 4MB from delaying w1, gpsimd: gb).
  - The PE p-state ramps 0.65->1.2->2.4 GHz over ~3us of continuous busy;
    a memset + 8 dummy matmuls at kernel start burn the ramp while the
    head DMA is still in flight, so real matmuls run at full clock.
"""

import numpy as np

B, N, D, E, F = 8, 512, 1024, 8, 2048
T = B * N
P = 128
CAP = 512            # per-expert token capacity = T/E (exact balance)
KT1 = D // P         # 8  k-tiles for matmul1 (contract over D)
MT1 = F // P         # 16 m-tiles for matmul1 / k-tiles for matmul2
MT2 = D // P         # 8  m-tiles for matmul2 (y^T rows)

W1_HEAD = 1                       # w1 m1-tiles packed into the head DMA
W1_BLOCKS = (1, 1, 2, 3, 4, 4)    # m1 = 1..15, fine-grained at the front
W2_BLK = 4                        # k2-tiles per w2 DMA
X_BLOCKS = (3, 4)                 # k = 1..7
N_DUMMY = 8                       # pre-ramp matmuls

_NC_CACHE = {}


def _build_bass():
    import concourse.bacc as bacc
    import concourse.tile as tile
    from concourse import mybir

    f32 = mybir.dt.float32
    bf16 = mybir.dt.bfloat16

    nc = bacc.Bacc(None, target_bir_lowering=False)
    # head: x k-tile 0 + w1 m1=0..W1_HEAD-1 column blocks in one transfer
    head = nc.declare_dram_parameter("head", [P, CAP + W1_HEAD * D], bf16,
                                     isOutput=False)
    gb = nc.declare_dram_parameter("gb", [P, MT1], f32, isOutput=False)
    # partition-major packs: row p holds that partition's full payload
    xb = nc.declare_dram_parameter("xb", [P, (KT1 - 1) * CAP], bf16,
                                   isOutput=False)
    w1b = nc.declare_dram_parameter("w1b", [P, (MT1 - W1_HEAD) * D], bf16,
                                    isOutput=False)
    w2b = nc.declare_dram_parameter("w2b", [P, MT1 * D], bf16,
                                    isOutput=False)
    yT = nc.declare_dram_parameter("yT", [D, CAP], bf16, isOutput=True)

    with tile.TileContext(nc) as tc:
        with (
            tc.tile_pool(name="gbp", bufs=1) as gbp,
            tc.tile_pool(name="hdp", bufs=1) as hdp,
            tc.tile_pool(name="dmp", bufs=1) as dmp,
            tc.tile_pool(name="xp", bufs=len(X_BLOCKS)) as xp,
            tc.tile_pool(name="w1p", bufs=len(W1_BLOCKS)) as w1p,
            tc.tile_pool(name="w2p", bufs=MT1 // W2_BLK) as w2p,
            tc.tile_pool(name="hp", bufs=MT1) as hp,
            tc.tile_pool(name="stp", bufs=4) as stp,
            tc.tile_pool(name="psA", bufs=3, space="PSUM") as psA,
            tc.tile_pool(name="psB", bufs=3, space="PSUM") as psB,
            tc.tile_pool(name="psD", bufs=1, space="PSUM") as psD,
        ):
            # ---- DMA issue -------------------------------------------
            # sync queue: head then x blocks (ring FIFO == arrival order)
            head_t = hdp.tile([P, CAP + W1_HEAD * D], bf16, tag="hd")
            nc.sync.dma_start(out=head_t[:], in_=head[:, :])
            x_blk, x_off = [], []
            off = 1
            for nk in X_BLOCKS:
                x_off.append(off)
                t = xp.tile([P, nk * CAP], bf16, tag="x",
                            name=f"x_{off}", padded_shape=[P, 4 * CAP])
                nc.sync.dma_start(
                    out=t[:], in_=xb[:, (off - 1) * CAP:(off - 1 + nk) * CAP])
                x_blk.append(t)
                off += nk

            def x_rhs(k):
                if k == 0:
                    return head_t[:, 0:CAP]
                j = next(i for i in range(len(X_BLOCKS))
                         if x_off[i] <= k < x_off[i] + X_BLOCKS[i])
                return x_blk[j][:, (k - x_off[j]) * CAP:(k - x_off[j] + 1) * CAP]

            # scalar queue: w1 m1=W1_HEAD..15 in growing blocks, THEN w2.
            w1_blk, w1_off = [], []
            off = W1_HEAD
            for nm in W1_BLOCKS:
                w1_off.append(off)
                t = w1p.tile([P, nm * D], bf16, tag="w1", name=f"w1_{off}",
                             padded_shape=[P, max(W1_BLOCKS) * D])
                nc.scalar.dma_start(
                    out=t[:], in_=w1b[:, (off - W1_HEAD) * D:
                                      (off - W1_HEAD + nm) * D])
                w1_blk.append(t)
                off += nm
            w2_sb = []
            for j in range(MT1 // W2_BLK):
                t = w2p.tile([P, W2_BLK * D], bf16, tag="w2", name=f"w2_{j}")
                nc.scalar.dma_start(
                    out=t[:], in_=w2b[:, j * W2_BLK * D:(j + 1) * W2_BLK * D])
                w2_sb.append(t)

            # gpsimd queue: bias columns
            gb_sb = gbp.tile([P, MT1], f32)
            nc.gpsimd.dma_start(out=gb_sb[:], in_=gb[:, :])

            def w1_lhs(m1, k):
                if m1 < W1_HEAD:
                    c0 = CAP + m1 * D + k * P
                    return head_t[:, c0:c0 + P]
                j = next(i for i in range(len(W1_BLOCKS))
                         if w1_off[i] <= m1 < w1_off[i] + W1_BLOCKS[i])
                c0 = (m1 - w1_off[j]) * D + k * P
                return w1_blk[j][:, c0:c0 + P]

            # ---- PE pre-ramp: burn the p-state ladder on junk ---------
            dum = dmp.tile([P, CAP], bf16, tag="dum")
            nc.vector.memset(dum[:], 0)
            psd = psD.tile([P, CAP], f32, tag="psD")
            for i in range(N_DUMMY):
                nc.tensor.matmul(psd[:], dum[:, 0:P], dum[:],
                                 start=True, stop=True, skip_group_check=True)

            # ---- Phase A: h^T[F, CAP] = gelu(w1^T @ x^T + b1) ----------
            gelu = mybir.ActivationFunctionType.Gelu
            h_sb = []
            for m1 in range(MT1):
                ps = psA.tile([P, CAP], f32, tag="psA", name=f"psA_{m1}")
                for k in range(KT1):
                    nc.tensor.matmul(ps[:], w1_lhs(m1, k), x_rhs(k),
                                     start=(k == 0), stop=(k == KT1 - 1))
                h = hp.tile([P, CAP], bf16, tag="h", name=f"h_{m1}")
                nc.scalar.activation(h[:], ps[:], gelu,
                                     bias=gb_sb[:, m1:m1 + 1])
                h_sb.append(h)

            # ---- Phase B: y^T[D, CAP] = w2^T @ h^T ---------------------
            # Last m-tile runs as two half-CAP groups so the final
            # copy+DMA chain after the very last matmul is half as long.
            def w2_lhs(k2, m):
                c0 = (k2 % W2_BLK) * D + m * P
                return w2_sb[k2 // W2_BLK][:, c0:c0 + P]

            for m in range(MT2 - 1):
                ps = psB.tile([P, CAP], f32, tag="psB", name=f"psB_{m}")
                for k2 in range(MT1):
                    nc.tensor.matmul(ps[:], w2_lhs(k2, m), h_sb[k2][:],
                                     start=(k2 == 0), stop=(k2 == MT1 - 1))
                stage = stp.tile([P, CAP], bf16, tag="st", name=f"st_{m}")
                nc.vector.tensor_scalar_mul(stage[:], ps[:], 1.0)
                nc.sync.dma_start(out=yT[m * P:(m + 1) * P, :],
                                  in_=stage[:])
            m = MT2 - 1
            for half, (a, b) in enumerate(((0, CAP // 2), (CAP // 2, CAP))):
                ps = psB.tile([P, b - a], f32, tag="psB", name=f"psB_{m}{half}")
                for k2 in range(MT1):
                    nc.tensor.matmul(ps[:], w2_lhs(k2, m), h_sb[k2][:, a:b],
                                     start=(k2 == 0), stop=(k2 == MT1 - 1))
                stage = stp.tile([P, b - a], bf16, tag="st",
                                 name=f"st_{m}{half}")
                nc.vector.tensor_scalar_mul(stage[:], ps[:], 1.0)
                nc.sync.dma_start(out=yT[m * P:(m + 1) * P, a:b],
                                  in_=stage[:])
    if not nc.is_finalized():
        nc.finalize()
    return nc


def _get_nc():
    if "nc" not in _NC_CACHE:
        _NC_CACHE["nc"] = _build_bass()
    return _NC_CACHE["nc"]


def kernel(x, dispatch_tensor, combine_tensor, w1, b1, w2, b2, **_):
    import ml_dtypes
    from concourse.bass_utils import run_bass_kernel_spmd

    bf = ml_dtypes.bfloat16
    x2d = np.ascontiguousarray(np.asarray(x, dtype=np.float32)).reshape(T, D)
    dispatch = np.asarray(dispatch_tensor, dtype=np.float32).reshape(T, E)
    combine = np.asarray(combine_tensor, dtype=np.float32).reshape(T, E)
    w1 = np.asarray(w1, dtype=np.float32)
    b1 = np.asarray(b1, dtype=np.float32)
    w2 = np.asarray(w2, dtype=np.float32)
    b2 = np.asarray(b2, dtype=np.float32)

    top = dispatch.argmax(-1)
    gate = combine.sum(-1)
    full = [np.nonzero(top == e)[0] for e in range(E)]
    idxs = [idx[:CAP] for idx in full]
    spill = [idx[CAP:] for idx in full]

    in_maps = []
    for e in range(E):
        idx = idxs[e]
        c = len(idx)
        xT = np.zeros((D, CAP), bf)
        xT[:, :c] = x2d[idx].T.astype(bf)
        # w1s[m1*P+p, k*P+m] = w1[k*P+p, m1*P+m]: per-m1 [P, D] blocks whose
        # [:, k*P:(k+1)*P] slice is the lhsT k-tile for output tile m1.
        w1s = np.ascontiguousarray(
            w1[e].reshape(KT1, P, MT1, P).transpose(2, 1, 0, 3)
        ).reshape(F, D).astype(bf)
        gbm = np.ascontiguousarray(b1[e].reshape(MT1, P).T)
        # partition-major packs: [n, P, w] -> [P, n*w]
        pm = lambda a: np.ascontiguousarray(
            a.transpose(1, 0, 2).reshape(P, -1))
        in_maps.append({
            "head": np.ascontiguousarray(np.concatenate(
                [xT[:P]] + [w1s[j * P:(j + 1) * P] for j in range(W1_HEAD)],
                axis=1)),
            "gb": gbm,
            "xb": pm(xT[P:].reshape(KT1 - 1, P, CAP)),
            "w1b": pm(w1s[W1_HEAD * P:].reshape(MT1 - W1_HEAD, P, D)),
            "w2b": pm(np.ascontiguousarray(w2[e]).astype(bf)
                      .reshape(MT1, P, D)),
        })

    global _LAST_IN_MAPS
    _LAST_IN_MAPS = in_maps
    nc = _get_nc()
    res = run_bass_kernel_spmd(nc, in_maps, list(range(E)))

    y_flat = np.empty((T, D), np.float32)
    for e in range(E):
        c = len(idxs[e])
        yTr = np.asarray(res.results[e]["yT"], dtype=np.float32)
        y_flat[idxs[e]] = yTr[:, :c].T * gate[idxs[e]][:, None]
        if len(spill[e]):
            # capacity-overflow fallback (exact fp32 math on host)
            import math

            erf = np.frompyfunc(math.erf, 1, 1)
            hs = x2d[spill[e]] @ w1[e] + b1[e]
            hs = hs * 0.5 * (1.0 + erf(hs / np.sqrt(2.0)).astype(np.float64))
            y_flat[spill[e]] = ((hs @ w2[e]) *
                                gate[spill[e]][:, None]).astype(np.float32)
    return (y_flat + b2[None, :]).reshape(B, N, D)
